# revision 2
# baseline (speedup 1.0000x reference)
"""DeformableInceptionModule on 8 axon-tunneled trn2 NeuronCores.

Three modulated deformable convs (3x3/5x5/7x7, DCNv2) over x[4,64,64,64],
outputs concatenated to [4,192,64,64].

On this stack the wall-clock is dominated by the axon tunnel: ~85ms fixed
dispatch floor per call and ~25-40MB/s host<->device bandwidth; device
execution itself is ~11ms/core. The design therefore minimizes round trips
and bytes on the wire:

  - 8-way shard (batch x H-half), one jit'd XLA program per core, all eight
    dispatched from concurrent threads (transfers/dispatch overlap ~4x).
  - Packed wire format per core (~2.2MB): offset integer parts as int8,
    fractional parts quantized to uint8 (1/256 px), masks/x as fp16,
    filters as bf16, output fetched as fp16. Validated rel-L2 vs the f32
    reference: ~0.0044 (gate is 2e-2).
  - Device program: quad-row gather table ([4356,256] bf16: channel vectors
    for all 4 bilinear neighbors in one row -> 512B DGE descriptors),
    coefficient chain with validity masks, one gather, one einsum combine,
    three bf16 GEMMs with f32 accumulation.

Persistent caches (/tmp/jax_pcc_deform, NUMBA_CACHE_DIR) make fresh-process
runs skip recompilation. If anything on the device path fails, a validated
single-core numba/numpy fallback produces the same result on host.
"""

import os
import numpy as np

os.environ.setdefault("NUMBA_CACHE_DIR", "/tmp/numba_cache_deform")

BRANCHES = [(3, 1, 9), (5, 2, 25), (7, 3, 49)]  # (ksize, pad, K)
KT = 83
NT = 2048   # 32 rows x 64 cols per core-shard
MAGIC = 12582912.0  # 1.5*2^23 round-to-nearest trick (numba fallback)

_JAX_STATE = {}


# --------------------------------------------------------------------------
# device path
# --------------------------------------------------------------------------

def _get_jax():
    """Lazy jax init; returns (jax, jnp, devices) or None."""
    if "res" in _JAX_STATE:
        return _JAX_STATE["res"]
    try:
        import jax
        try:
            jax.config.update("jax_compilation_cache_dir", "/tmp/jax_pcc_deform")
            jax.config.update("jax_persistent_cache_min_entry_size_bytes", 0)
            jax.config.update("jax_persistent_cache_min_compile_time_secs", 0)
        except Exception:
            pass
        import jax.numpy as jnp
        devs = jax.devices()
        if len(devs) < 8:
            _JAX_STATE["res"] = None
            return None
        _JAX_STATE["res"] = (jax, jnp, devs)
        return _JAX_STATE["res"]
    except Exception:
        _JAX_STATE["res"] = None
        return None


def _build_device_fn(jax, jnp):
    def device_fn(pk, m16, x16, h0v, W3a, W3b, W3c):
        # pk [4, KT, NT] uint8; m16 [KT, NT] f16; x16 [64, 4096] f16;
        # h0v [1] f32; W3* [K, 64, 64] bf16
        pkf = pk.astype(jnp.float32)
        ey = pkf[0] - 5.0
        ex = pkf[1] - 5.0
        fy = pkf[2] * (1.0 / 256.0)
        fx = pkf[3] * (1.0 / 256.0)

        n = jnp.arange(NT, dtype=jnp.float32)
        hh = h0v[0] + jnp.floor(n * (1.0 / 64.0))
        ww = jnp.mod(n, 64.0)
        kys = []
        kxs = []
        for (ks, pad, K) in BRANCHES:
            ky, kx = jnp.meshgrid(jnp.arange(ks, dtype=jnp.float32),
                                  jnp.arange(ks, dtype=jnp.float32),
                                  indexing="ij")
            kys.append(ky.reshape(K) - pad)
            kxs.append(kx.reshape(K) - pad)
        kyv = jnp.concatenate(kys)
        kxv = jnp.concatenate(kxs)
        y0 = (hh[None, :] + kyv[:, None]) + ey     # [KT, NT]
        x0 = (ww[None, :] + kxv[:, None]) + ex

        vy0 = ((y0 >= 0) & (y0 <= 63)).astype(jnp.float32)
        vy1 = ((y0 >= -1) & (y0 <= 62)).astype(jnp.float32)
        vx0 = ((x0 >= 0) & (x0 <= 63)).astype(jnp.float32)
        vx1 = ((x0 >= -1) & (x0 <= 62)).astype(jnp.float32)
        mf = m16.astype(jnp.float32)
        wy1 = mf * fy
        wy0 = mf - wy1
        cy0 = wy0 * vy0
        cy1 = wy1 * vy1
        cx0 = (1.0 - fx) * vx0
        cx1 = fx * vx1
        coef = jnp.stack([cy0 * cx0, cy0 * cx1, cy1 * cx0, cy1 * cx1],
                         axis=-1).astype(jnp.bfloat16)     # [KT, NT, 4]

        y0c = jnp.clip(y0, -1.0, 64.0)
        x0c = jnp.clip(x0, -1.0, 64.0)
        pos = ((y0c + 1.0) * 66.0 + (x0c + 1.0)).astype(jnp.int32)

        xT3 = x16.astype(jnp.float32).T.reshape(64, 64, 64)
        xp = jnp.pad(xT3, ((1, 2), (1, 2), (0, 0)))
        Tq = jnp.concatenate(
            [xp[0:66, 0:66], xp[0:66, 1:67], xp[1:67, 0:66], xp[1:67, 1:67]],
            axis=-1)
        Tq = Tq.reshape(66 * 66, 256).astype(jnp.bfloat16)

        g = jnp.take(Tq, pos.reshape(-1), axis=0).reshape(KT, NT, 4, 64)
        samp = jnp.einsum("knqc,knq->knc", g, coef,
                          preferred_element_type=jnp.bfloat16)

        outs = []
        k0 = 0
        for (K, W3) in ((9, W3a), (25, W3b), (49, W3c)):
            s = jax.lax.slice_in_dim(samp, k0, k0 + K, axis=0)
            k0 += K
            o = jax.lax.dot_general(
                s, W3,
                dimension_numbers=(((0, 2), (0, 1)), ((), ())),
                preferred_element_type=jnp.float32)     # [NT, 64]
            outs.append(o)
        return jnp.concatenate(outs, axis=1).astype(jnp.float16)  # [NT, 192]

    return jax.jit(device_fn)


def _pack_core(x, offs, masks, b, h0):
    """Host-side packed shard for one core."""
    dy = np.concatenate(
        [o[b, 0::2, h0:h0 + 32, :].reshape(-1, NT) for o in offs], 0)
    dx = np.concatenate(
        [o[b, 1::2, h0:h0 + 32, :].reshape(-1, NT) for o in offs], 0)
    m = np.concatenate(
        [mk[b, :, h0:h0 + 32, :].reshape(-1, NT) for mk in masks], 0)
    fldy = np.floor(dy)
    fldx = np.floor(dx)
    pk = np.empty((4, KT, NT), np.uint8)
    pk[0] = (np.clip(fldy, -5, 4) + 5.0).astype(np.uint8)
    pk[1] = (np.clip(fldx, -5, 4) + 5.0).astype(np.uint8)
    pk[2] = np.clip(np.round((dy - fldy) * 256.0), 0, 255).astype(np.uint8)
    pk[3] = np.clip(np.round((dx - fldx) * 256.0), 0, 255).astype(np.uint8)
    return pk, m.astype(np.float16)


def _kernel_device(x, filts, offs, masks):
    res = _get_jax()
    if res is None:
        raise RuntimeError("no jax/devices")
    jax, jnp, devs = res
    import ml_dtypes
    if "fn" not in _JAX_STATE:
        _JAX_STATE["fn"] = _build_device_fn(jax, jnp)
    fn = _JAX_STATE["fn"]

    W3 = []
    for j, (ks, pad, K) in enumerate(BRANCHES):
        w = filts[j].reshape(64, 64, K)
        W3.append(np.ascontiguousarray(
            np.transpose(w, (2, 1, 0))).astype(ml_dtypes.bfloat16))
    x16s = [np.ascontiguousarray(x[b].reshape(64, 4096)).astype(np.float16)
            for b in range(4)]

    outs = [None] * 8
    errs = []

    def run(core):
        try:
            b, half = core // 2, core % 2
            h0 = 32 * half
            pk, m16 = _pack_core(x, offs, masks, b, h0)
            d = devs[core]
            args = (
                jax.device_put(pk, d),
                jax.device_put(m16, d),
                jax.device_put(x16s[b], d),
                jax.device_put(np.array([h0], np.float32), d),
                jax.device_put(W3[0], d),
                jax.device_put(W3[1], d),
                jax.device_put(W3[2], d),
            )
            r = fn(*args)
            outs[core] = np.asarray(r)
        except Exception as e:  # noqa: BLE001
            errs.append(e)

    from concurrent.futures import ThreadPoolExecutor
    with ThreadPoolExecutor(max_workers=8) as ex:
        list(ex.map(run, range(8)))
    if errs:
        raise errs[0]

    full = np.zeros((4, 192, 64, 64), np.float32)
    for core in range(8):
        b, half = core // 2, core % 2
        full[b, :, 32 * half:32 * half + 32, :] = (
            outs[core].astype(np.float32).T.reshape(192, 32, 64))
    return full


# --------------------------------------------------------------------------
# public entry
# --------------------------------------------------------------------------

def kernel(x, filter1, offset1, mask1, filter2, offset2, mask2,
           filter3, offset3, mask3):
    x = np.asarray(x, dtype=np.float32)
    filts = [np.asarray(filter1, np.float32), np.asarray(filter2, np.float32),
             np.asarray(filter3, np.float32)]
    offs = [np.asarray(offset1, np.float32), np.asarray(offset2, np.float32),
            np.asarray(offset3, np.float32)]
    masks = [np.asarray(mask1, np.float32), np.asarray(mask2, np.float32),
             np.asarray(mask3, np.float32)]
    try:
        return _kernel_device(x, filts, offs, masks)
    except Exception:
        return _kernel_numpy(x, filts, offs, masks)


# --------------------------------------------------------------------------
# host fallback (exact algorithm, validated vs reference)
# --------------------------------------------------------------------------

def _kernel_numpy(x, filts, offs, masks):
    full = np.zeros((4, 192, 64, 64), np.float32)
    for b in range(4):
        full[b] = _np_batch(x, filts, offs, masks, b).reshape(192, 64, 64)
    return full


def _np_batch(x, filts, offs, masks, b):
    NTF = 4096
    dy = np.concatenate([o[b, 0::2].reshape(-1, NTF) for o in offs], 0)
    dx = np.concatenate([o[b, 1::2].reshape(-1, NTF) for o in offs], 0)
    m = np.concatenate([mk[b].reshape(-1, NTF) for mk in masks], 0)
    n = np.arange(NTF)
    HG = np.zeros((KT, NTF), np.float32)
    WG = np.zeros((KT, NTF), np.float32)
    wblk = np.zeros((KT, 64, 64), np.float32)
    kg = 0
    for j, (ks, pad, K) in enumerate(BRANCHES):
        wj = filts[j].reshape(64, 64, K)
        for kl in range(K):
            ky, kx = kl // ks, kl % ks
            HG[kg] = (n // 64) + (ky - pad)
            WG[kg] = (n % 64) + (kx - pad)
            wblk[kg] = wj[:, :, kl].T
            kg += 1
    xT = x[b].reshape(64, NTF).astype(np.float32).T
    xT2 = np.zeros((4288, 128), np.float32)
    xT2[65:4161, 0:64] = xT
    xT2[64:4160, 64:128] = xT
    py = dy + HG
    y0f = (py - 0.5 + MAGIC) - MAGIC
    wy = py - y0f
    px = dx + WG
    x0f = (px - 0.5 + MAGIC) - MAGIC
    wx = px - x0f
    vy0 = ((y0f >= 0.0) & (y0f <= 63.0)).astype(np.float32)
    vy1 = ((y0f >= -1.0) & (y0f <= 62.0)).astype(np.float32)
    vx0 = ((x0f >= 0.0) & (x0f <= 63.0)).astype(np.float32)
    vx1 = ((x0f >= -1.0) & (x0f <= 62.0)).astype(np.float32)
    mw = m * wy
    m0 = m - mw
    wyf0 = m0 * vy0; wyf1 = mw * vy1
    wxf0 = (1.0 - wx) * vx0; wxf1 = wx * vx1
    c00 = wyf0 * wxf0; c01 = wyf0 * wxf1
    c10 = wyf1 * wxf0; c11 = wyf1 * wxf1
    pos = (np.clip(y0f, -1.0, 63.0) * 64.0
           + np.clip(x0f + 65.0, 64.0, 128.0)).astype(np.intp)

    out = np.empty((192, NTF), np.float32)
    NB = 128
    Kmax = max(K for (_, _, K) in BRANCHES)
    samp = np.empty((Kmax, NB, 64), np.float32)
    tmp = np.empty((Kmax, NB, 64), np.float32)
    A = np.empty((Kmax * 64, NB), np.float32)
    fused = _get_fused()
    k0 = 0
    for ji, (ks, pad, K) in enumerate(BRANCHES):
        kk0, kk1 = k0, k0 + K
        k0 += K
        Wm = wblk[kk0:kk1].reshape(K * 64, 64)
        s = samp[:K]; t = tmp[:K]; Av = A[:K * 64]
        ob = out[ji * 64:(ji + 1) * 64]
        posb = pos[kk0:kk1]
        cb00 = c00[kk0:kk1]; cb01 = c01[kk0:kk1]
        cb10 = c10[kk0:kk1]; cb11 = c11[kk0:kk1]
        for n0 in range(0, NTF, NB):
            if fused is not None:
                fused(xT2, posb, cb00, cb01, cb10, cb11, s, n0, NB, K)
            else:
                nsl = slice(n0, n0 + NB)
                p0 = posb[:, nsl]
                g0 = xT2[p0]
                g1 = xT2[p0 + 64]
                np.multiply(g0[:, :, 0:64], cb00[:, nsl, None], out=s)
                np.multiply(g0[:, :, 64:128], cb01[:, nsl, None], out=t)
                s += t
                np.multiply(g1[:, :, 0:64], cb10[:, nsl, None], out=t)
                s += t
                np.multiply(g1[:, :, 64:128], cb11[:, nsl, None], out=t)
                s += t
            Av[:] = s.transpose(0, 2, 1).reshape(K * 64, NB)
            np.matmul(Wm.T, Av, out=ob[:, n0:n0 + NB])
    return out


_FUSED = None


def _get_fused():
    global _FUSED
    if _FUSED is not None:
        return _FUSED if _FUSED is not False else None
    try:
        from numba import njit

        @njit(cache=True, fastmath=False)
        def fused(xT2, pos, c00, c01, c10, c11, samp, n0, NB, K):
            for k in range(K):
                for n in range(NB):
                    r0 = pos[k, n0 + n]
                    a = c00[k, n0 + n]; b = c01[k, n0 + n]
                    c = c10[k, n0 + n]; d = c11[k, n0 + n]
                    for ch in range(64):
                        samp[k, n, ch] = (
                            xT2[r0, ch] * a + xT2[r0, 64 + ch] * b
                            + xT2[r0 + 64, ch] * c + xT2[r0 + 64, 64 + ch] * d)

        _FUSED = fused
        return fused
    except Exception:
        _FUSED = False
        return None
